# revision 5
# baseline (speedup 1.0000x reference)
"""Trainium2 Bass kernel for nn_HadamardExpansionV2 (topk_masking).

Reference computation:
  mask  = hard gumbel-softmax over c1=256, for 2*ce rows  -> numerically an
          exact one-hot matrix scaled by w=(1-s)+s (w==1.0 in fp32 here)
  x_i   = einsum('ec,bcl->bel', mask[0], x)   == gather of channels i0[e]
  x_j   = einsum('ec,bcl->bel', mask[1], x)   == gather of channels i1[e]
  xe    = x_i * x_j                            [B, ce, H, W]
  out   = BatchNorm2d(train mode, batch stats over (B,H,W)) * gamma + beta

Strategy (8 NeuronCores, no collectives), v2:
  - Shard ce=512 across cores (64 e's each); BN stats fully core-local.
  - Host pre-gathers the channel pairs into xsel [128, B*L] int8 with exact
    per-channel-row scales. Device computes RAW int8 products prod_q =
    xi_q*xj_q in f16; all dequant scales are folded into the per-partition
    BN affine (A, nbneg), so only ONE product pass is needed:
        out = A*prod_q + nbneg
        A    = gamma * rsqrt(var_q + eps/(gamma_w_s)^2)   [per partition]
        nbneg= beta - A*mean_q
  - Engine balance per group g of 8 e's (partition p = (e_sub, b)):
      SP   in-DMA (all 8 groups issued upfront, bufs=8)
      DVE  STT prod_q (1x, paces) + S accum; negvar; recip; negA; norm
           on the first LD columns (4x f16 TS)
      ACT  Square+SS accum; agg copy; Sqrt; nbneg (Identity, AP scale+bias);
           norm on the next LA columns (Identity with per-partition A, nbneg)
      PE   (R R^T)/N matmul -> (mean, ssn) replicated over the e-block
      POOL norm on the last LP columns (tensor_scalar) + both out-DMAs
           (SWDGE) so the ACT sequencer never parks on store sems
    Square/Sqrt/Identity all live in the 'sqrt_and_others' act table ->
    exactly one ACT table load for the whole kernel.
  - Head taper: group 0's load and STT are split into 4 column chunks so the
    first DVE work starts ~4us earlier.
  - Tail taper: group 7's STT is split into 4 chunks over two half-width
    prod tiles, Square into 2 halves, so its stats chain closes ~2.5us after
    the last chunk; its norms/stores run as two half-width pieces.

The bass program depends only on shapes -> compiled once and cached.
"""

import os
import sys
from contextlib import ExitStack

import numpy as np

sys.path.insert(0, "/opt/trn_rl_repo")

import concourse.bass as bass  # noqa: E402
import concourse.tile as tile  # noqa: E402
import concourse.mybir as mybir  # noqa: E402
from concourse import bacc  # noqa: E402
from concourse.bass_utils import run_bass_kernel_spmd  # noqa: E402

# Problem shapes (hardcoded per contract)
B, C1, H, W = 16, 256, 56, 56
L = H * W                      # 3136
CE = 512
NCORES = 8
EPC = CE // NCORES             # 64 e-channels per core
NG = 8                         # groups per core
EG = EPC // NG                 # 8 e-channels per group
N = B * L                      # 50176 elements per channel for BN stats
BN_EPS = 1e-5

F32 = mybir.dt.float32
F16 = mybir.dt.float16
I8 = mybir.dt.int8

NCOEF = 4                      # coef cols per group: -1/g^2, eps', beta, 0

GATHER_DTYPE = os.environ.get("KERNEL_GATHER_DTYPE", "i8")
OUT_DTYPE = os.environ.get("KERNEL_OUT_DTYPE", "f16")

# norm column split across engines (fractions of L)
FD = float(os.environ.get("KERNEL_FD", "0.53"))   # DVE share
FP = float(os.environ.get("KERNEL_FP", "0.28"))   # Pool share (0 = off)
FD7 = float(os.environ.get("KERNEL_FD7", "0.72"))  # DVE share, tail halves

CH = 4                         # head/tail column chunks
LCH = L // CH                  # 784
LH = L // 2                    # 1568

_PROGRAMS = {}
LAST_RESULT = None


def _al8(x):
    return (int(x) // 8) * 8


def _build_program(gdt_name, odt_name):
    gdt = {"f16": F16, "i8": I8, "f32": F32}[gdt_name]
    odt = F16 if odt_name == "f16" else F32
    nc = bacc.Bacc("TRN2", target_bir_lowering=False, debug=False,
                   num_devices=NCORES)

    AF = mybir.ActivationFunctionType
    OP = mybir.AluOpType

    LD = _al8(FD * L)
    LP = _al8(FP * L)
    LA = L - LD - LP
    assert LA > 0
    LD7 = _al8(FD7 * LH)
    LA7 = LH - LD7

    xsel_d = nc.dram_tensor("xsel", [128, N], gdt, kind="ExternalInput").ap()
    coef_d = nc.dram_tensor("coef", [128, NCOEF * NG], F32,
                            kind="ExternalInput").ap()
    rr_d = nc.dram_tensor("rr", [128, 128], F32, kind="ExternalInput").ap()
    out_d = nc.dram_tensor("out", [EPC, B, L], odt, kind="ExternalOutput").ap()

    # per-group input view: [g, (e b), m, l];  src row (m*64+g*8+e), col (b,l)
    xsel_r = xsel_d.rearrange("(m g e) (b l) -> g (e b) m l", m=2, g=NG, b=B)
    out_r = out_d.rearrange("(g e) b l -> g (e b) l", g=NG)

    with tile.TileContext(nc) as tc, ExitStack() as ctx:
        const_pool = ctx.enter_context(tc.tile_pool(name="consts", bufs=1))
        xioc_pool = ctx.enter_context(tc.tile_pool(name="xioc", bufs=CH))
        xio_pool = ctx.enter_context(tc.tile_pool(name="xio", bufs=7))
        prod_pool = ctx.enter_context(tc.tile_pool(name="prod", bufs=7))
        prod7_pool = ctx.enter_context(tc.tile_pool(name="prod7", bufs=2))
        sq_pool = ctx.enter_context(tc.tile_pool(name="sq", bufs=2))
        stats_pool = ctx.enter_context(tc.tile_pool(name="stats", bufs=4))
        sm_pool = ctx.enter_context(tc.tile_pool(name="smalls", bufs=4))
        outd_pool = ctx.enter_context(tc.tile_pool(name="outd", bufs=3))
        outap_pool = ctx.enter_context(tc.tile_pool(name="outap", bufs=3))
        out7_pool = ctx.enter_context(tc.tile_pool(name="out7", bufs=2))
        psum_pool = ctx.enter_context(
            tc.tile_pool(name="psum", bufs=4, space="PSUM"))

        # ---- constants on the ACT ring (its first two instructions) ----
        coef_sb = const_pool.tile([128, NCOEF * NG], F32)
        nc.scalar.dma_start(coef_sb[:], coef_d[:])
        rr_sb = const_pool.tile([128, 128], F32)
        nc.scalar.dma_start(rr_sb[:], rr_d[:])

        def cf(g, c):
            return coef_sb[:, NCOEF * g + c:NCOEF * g + c + 1]

        # ---- all input loads upfront on the SP ring ----
        xioc = [xioc_pool.tile([128, 2 * LCH], gdt, tag="xioc", name=f"xioc{c}")
                for c in range(CH)]
        for c in range(CH):
            dst = xioc[c][:].rearrange("p (m l) -> p m l", m=2)
            nc.sync.dma_start(dst, xsel_r[0][:, :, c * LCH:(c + 1) * LCH])
        xio = [None] * NG
        for g in range(1, NG):
            xio[g] = xio_pool.tile([128, 2 * L], gdt, tag="xio", name=f"xio{g}")
            dst = xio[g][:].rearrange("p (m l) -> p m l", m=2)
            nc.sync.dma_start(dst, xsel_r[g])

        # ---- per-group state ----
        prod = [None] * NG
        prod7 = [None, None]
        stats = [None] * NG
        agg = [None] * NG
        sm = [None] * NG

        # sm cols: 0=mean 1=ssn 2=negvar 3=sd 4=A 5=negA 6=nbneg
        def make_sm(g):
            sm[g] = sm_pool.tile([128, 7], F32, tag="sm", name=f"sm{g}")

        def stt(g):
            prod[g] = prod_pool.tile([128, L], F16, tag="prod", name=f"prod{g}")
            stats[g] = stats_pool.tile([128, 2], F32, tag="st", name=f"st{g}")
            nc.vector.scalar_tensor_tensor(
                out=prod[g][:], in0=xio[g][:, 0:L], scalar=1.0,
                in1=xio[g][:, L:2 * L],
                op0=OP.mult, op1=OP.mult,
                accum_out=stats[g][:, 0:1])

        def stt0_chunk(c):
            if prod[0] is None:
                prod[0] = prod_pool.tile([128, L], F16, tag="prod",
                                         name="prod0")
                # cols: S0..3 @0:4, tmp @4:6, S @6, SS @7
                stats[0] = stats_pool.tile([128, 8], F32, tag="st0",
                                           name="st0")
            cs = slice(c * LCH, (c + 1) * LCH)
            nc.vector.scalar_tensor_tensor(
                out=prod[0][:, cs], in0=xioc[c][:, 0:LCH], scalar=1.0,
                in1=xioc[c][:, LCH:2 * LCH],
                op0=OP.mult, op1=OP.mult,
                accum_out=stats[0][:, c:c + 1])

        def pre0():
            st = stats[0]
            nc.vector.tensor_tensor(out=st[:, 4:6], in0=st[:, 0:2],
                                    in1=st[:, 2:4], op=OP.add)
            nc.vector.tensor_tensor(out=st[:, 6:7], in0=st[:, 4:5],
                                    in1=st[:, 5:6], op=OP.add)

        def stt7_chunk(c):
            if prod7[0] is None:
                prod7[0] = prod7_pool.tile([128, LH], F16, tag="prod7", name="prod7a")
                prod7[1] = prod7_pool.tile([128, LH], F16, tag="prod7", name="prod7b")
                # cols: S0..3 @0:4, SSa @4, SSb @5, tmp @6:8, S @8, SS @9
                stats[7] = stats_pool.tile([128, 10], F32, tag="st7",
                                           name="st7")
            h, hc = divmod(c, 2)
            cs = slice(hc * LCH, (hc + 1) * LCH)
            src0 = xio[7][:, c * LCH:(c + 1) * LCH]
            src1 = xio[7][:, L + c * LCH:L + (c + 1) * LCH]
            nc.vector.scalar_tensor_tensor(
                out=prod7[h][:, cs], in0=src0, scalar=1.0, in1=src1,
                op0=OP.mult, op1=OP.mult,
                accum_out=stats[7][:, c:c + 1])

        def pre7():
            st = stats[7]
            nc.vector.tensor_tensor(out=st[:, 6:8], in0=st[:, 0:2],
                                    in1=st[:, 2:4], op=OP.add)
            nc.vector.tensor_tensor(out=st[:, 8:9], in0=st[:, 6:7],
                                    in1=st[:, 7:8], op=OP.add)
            nc.vector.tensor_tensor(out=st[:, 9:10], in0=st[:, 4:5],
                                    in1=st[:, 5:6], op=OP.add)

        sqt = [None, None]

        def square(g):
            t = sq_pool.tile([128, L], F32, tag="sq", name=f"sqt{g}")
            if g == 0:
                nc.scalar.activation(out=t[:], in_=prod[0][:], func=AF.Square,
                                     accum_out=stats[0][:, 7:8])
            else:
                nc.scalar.activation(out=t[:], in_=prod[g][:], func=AF.Square,
                                     accum_out=stats[g][:, 1:2])

        def square7(h):
            if sqt[0] is None:
                sqt[0] = sq_pool.tile([128, L], F32, tag="sq", name="sqt7")
            cs = slice(h * LH, (h + 1) * LH)
            nc.scalar.activation(out=sqt[0][:, cs], in_=prod7[h][:],
                                 func=AF.Square,
                                 accum_out=stats[7][:, 4 + h:5 + h])

        def mm(g):
            agg[g] = psum_pool.tile([128, 2], F32, tag="agg", name=f"agg{g}")
            sc = {0: 6, 7: 8}.get(g, 0)
            nc.tensor.matmul(agg[g][:], rr_sb[:], stats[g][:, sc:sc + 2],
                             start=True, stop=True)

        def chain_cp(g):
            # ACT: (mean, ssn) PSUM -> SBUF
            make_sm(g)
            nc.scalar.activation(out=sm[g][:, 0:2], in_=agg[g][:],
                                 func=AF.Identity)

        def chain_nv(g):
            # DVE: negvar = mean*mean - ssn
            nc.vector.tensor_scalar(out=sm[g][:, 2:3], in0=sm[g][:, 0:1],
                                    scalar1=sm[g][:, 0:1],
                                    scalar2=sm[g][:, 1:2],
                                    op0=OP.mult, op1=OP.subtract)

        def chain_sqrt(g):
            # ACT: sd = Sqrt(negvar*(-1/g^2) + eps'/g^2) = sqrt(var+eps')/gamma
            nc.scalar.activation(out=sm[g][:, 3:4], in_=sm[g][:, 2:3],
                                 func=AF.Sqrt,
                                 scale=cf(g, 0), bias=cf(g, 1))

        def chain_rcp(g):
            # DVE: A = 1/sd ; negA = -A
            nc.vector.reciprocal(sm[g][:, 4:5], sm[g][:, 3:4])
            nc.vector.tensor_scalar(out=sm[g][:, 5:6], in0=sm[g][:, 4:5],
                                    scalar1=-1.0, scalar2=None, op0=OP.mult)

        def chain_nbneg(g):
            # ACT: nbneg = mean*negA + beta
            nc.scalar.activation(out=sm[g][:, 6:7], in_=sm[g][:, 0:1],
                                 func=AF.Identity,
                                 scale=sm[g][:, 5:6], bias=cf(g, 2))

        outd = [None] * NG
        outap = [None] * NG
        out7t = [None, None]

        def normD(g):
            outd[g] = outd_pool.tile([128, LD], odt, tag="outd", name=f"outd{g}")
            nc.vector.tensor_scalar(out=outd[g][:], in0=prod[g][:, 0:LD],
                                    scalar1=sm[g][:, 4:5],
                                    scalar2=sm[g][:, 6:7],
                                    op0=OP.mult, op1=OP.add)

        def normA(g):
            outap[g] = outap_pool.tile([128, LA + LP], odt, tag="outap", name=f"outap{g}")
            nc.scalar.activation(out=outap[g][:, 0:LA],
                                 in_=prod[g][:, LD:LD + LA],
                                 func=AF.Identity,
                                 scale=sm[g][:, 4:5], bias=sm[g][:, 6:7])

        def normP(g):
            if LP == 0:
                return
            nc.gpsimd.tensor_scalar(out=outap[g][:, LA:LA + LP],
                                    in0=prod[g][:, LD + LA:L],
                                    scalar1=sm[g][:, 4:5],
                                    scalar2=sm[g][:, 6:7],
                                    op0=OP.mult, op1=OP.add)

        def outDdma(g):
            nc.gpsimd.dma_start(out_r[g][:, 0:LD], outd[g][:])

        def outAPdma(g):
            nc.gpsimd.dma_start(out_r[g][:, LD:L], outap[g][:])

        def norm7(h):
            out7t[h] = out7_pool.tile([128, LH], odt, tag="out7", name=f"out7t{h}")
            cs0 = slice(h * LH, h * LH + LD7)
            nc.vector.tensor_scalar(out=out7t[h][:, 0:LD7],
                                    in0=prod7[h][:, 0:LD7],
                                    scalar1=sm[7][:, 4:5],
                                    scalar2=sm[7][:, 6:7],
                                    op0=OP.mult, op1=OP.add)
            nc.scalar.activation(out=out7t[h][:, LD7:LH],
                                 in_=prod7[h][:, LD7:LH],
                                 func=AF.Identity,
                                 scale=sm[7][:, 4:5], bias=sm[7][:, 6:7])
            nc.gpsimd.dma_start(out_r[7][:, h * LH:(h + 1) * LH],
                                out7t[h][:])

        def chain(g):
            chain_cp(g)
            chain_nv(g)
            chain_sqrt(g)
            chain_rcp(g)
            chain_nbneg(g)

        # ---- schedule ----
        # head: g0 chunked
        for c in range(CH):
            stt0_chunk(c)
        pre0()
        square(0)
        mm(0)
        stt(1)
        chain(0)
        stt(2)
        square(1)
        mm(1)
        chain(1)
        normD(0); normA(0); normP(0)
        outDdma(0); outAPdma(0)
        for g in range(3, 7):
            stt(g)
            square(g - 1)
            mm(g - 1)
            chain(g - 1)
            normD(g - 2); normA(g - 2); normP(g - 2)
            outDdma(g - 2); outAPdma(g - 2)
        # g7 head chunks while g6 stats close
        stt7_chunk(0)
        stt7_chunk(1)
        square(6)
        mm(6)
        chain(6)
        normD(5); normA(5); normP(5)
        outDdma(5); outAPdma(5)
        stt7_chunk(2)
        square7(0)
        stt7_chunk(3)
        square7(1)
        pre7()
        mm(7)
        chain(7)
        normD(6); normA(6); normP(6)
        outDdma(6); outAPdma(6)
        norm7(0)
        norm7(1)

    nc.compile()
    return nc


def _get_program(gdt_name=None, odt_name=None):
    gdt_name = gdt_name or GATHER_DTYPE
    odt_name = odt_name or OUT_DTYPE
    key = (gdt_name, odt_name)
    if key not in _PROGRAMS:
        _PROGRAMS[key] = _build_program(gdt_name, odt_name)
    return _PROGRAMS[key]


def _host_prep(x, logits, gumbel, tau, gamma, beta):
    """Compute mask indices/weights and build per-core inputs."""
    x = np.asarray(x, dtype=np.float32)
    logits = np.asarray(logits, dtype=np.float32)
    gumbel = np.asarray(gumbel, dtype=np.float32)
    tau_f = np.float32(np.asarray(tau))
    gamma = np.asarray(gamma, dtype=np.float32)
    beta = np.asarray(beta, dtype=np.float32)

    # replicate reference softmax/argmax in fp32 (argmax of z == argmax of
    # softmax(z); min top-2 gap 3.4e-4 >> fp32 eps for these inputs)
    z = (logits + gumbel) / tau_f                     # [2, CE, C1]
    idx = z.argmax(axis=-1)                           # [2, CE]
    zm = z.max(axis=-1, keepdims=True)
    ez = np.exp(z - zm, dtype=np.float32)
    soft = ez / ez.sum(axis=-1, keepdims=True, dtype=np.float32)
    s_hot = np.take_along_axis(soft, idx[..., None], axis=-1)[..., 0]
    w = (np.float32(1.0) - s_hot) + s_hot             # [2, CE] (== 1.0 here)
    weff = (w[0] * w[1]).astype(np.float32)           # [CE]

    xt = np.ascontiguousarray(
        x.reshape(B, C1, L).transpose(1, 0, 2)).reshape(C1, N)
    if GATHER_DTYPE == "f16":
        xq = xt.astype(np.float16)
        xscale = np.ones((C1,), dtype=np.float32)
    elif GATHER_DTYPE == "i8":
        xscale = (np.abs(xt).max(axis=1) / np.float32(127.0)).astype(np.float32)
        xq = np.rint(xt / xscale[:, None]).astype(np.int8)
    else:
        xq = xt
        xscale = np.ones((C1,), dtype=np.float32)

    # RR^T/N: block one-hot outer product (partition p in e-block p//B)
    rr = np.zeros((128, 128), dtype=np.float32)
    inv_n = np.float32(1.0) / np.float32(N)
    for es in range(EG):
        rr[es * B:(es + 1) * B, es * B:(es + 1) * B] = inv_n

    in_maps = []
    for k in range(NCORES):
        e0 = k * EPC
        rows = np.concatenate([idx[0, e0:e0 + EPC], idx[1, e0:e0 + EPC]])
        xsel = np.ascontiguousarray(xq[rows])         # [128, N]

        coef = np.zeros((128, NCOEF * NG), dtype=np.float32)
        p = np.arange(128)
        for g in range(NG):
            el = e0 + g * EG + p // B                 # global e per partition
            wv = weff[el]
            gm = gamma[el]
            assert np.all(gm * wv > 0), "fold assumes gamma*w > 0"
            s = (xscale[idx[0, el]] * xscale[idx[1, el]]).astype(np.float32)
            # sd = Sqrt(negvar * c0 + c1);  A = 1/sd = gamma*rstd_q
            coef[:, NCOEF * g + 0] = -1.0 / (gm * gm)
            coef[:, NCOEF * g + 1] = (np.float32(BN_EPS)
                                      / np.square(gm * wv * s))
            coef[:, NCOEF * g + 2] = beta[el]

        in_maps.append({"xsel": xsel, "coef": coef, "rr": rr})
    return in_maps


def _install_ntff_shim():
    """The agent image's antenv lacks axon_hooks; recreate it so
    run_bass_kernel_spmd(trace=True) can capture NTFF profiles."""
    import types
    if "antenv.axon_hooks" in sys.modules:
        return
    mod = types.ModuleType("antenv.axon_hooks")
    _hook = [None]
    mod.set_axon_ntff_profile_hook = lambda h: _hook.__setitem__(0, h)
    mod.get_axon_ntff_profile_hook = lambda: _hook[0]
    sys.modules["antenv.axon_hooks"] = mod
    import antenv
    antenv.axon_hooks = mod
    from trn_agent_boot.trn_boot import _ntff_profile_via_ctypes
    mod.set_axon_ntff_profile_hook(
        _ntff_profile_via_ctypes("/opt/axon/libaxon_pjrt.so"))


def kernel(x, logits, gumbel, tau, gamma, beta):
    global LAST_RESULT
    nc = _get_program()
    in_maps = _host_prep(x, logits, gumbel, tau, gamma, beta)

    trace = bool(int(os.environ.get("KERNEL_PROFILE", "0")))
    if trace:
        try:
            _install_ntff_shim()
        except Exception:
            trace = False
    try:
        res = run_bass_kernel_spmd(nc, in_maps, list(range(NCORES)),
                                   trace=trace)
    except Exception:
        if not trace:
            raise
        res = run_bass_kernel_spmd(nc, in_maps, list(range(NCORES)),
                                   trace=False)
    LAST_RESULT = res

    out = np.empty((B, CE, L), dtype=np.float32)
    for k in range(NCORES):
        out[:, k * EPC:(k + 1) * EPC, :] = res.results[k]["out"].transpose(1, 0, 2)
    return out.reshape(B, CE, H, W)
